# revision 22
# baseline (speedup 1.0000x reference)
"""Distributed 2-layer GAT kernel for 8 Trainium2 NeuronCores.

Strategy (host graph preprocessing + device SPMD kernel):
  * Nodes are relabeled by in-degree (ascending) and padded to 20480 ids.
    Blocks of 128 consecutive ids then have near-uniform in-degree, and the
    160 blocks are dealt round-robin to the 8 cores (core = block % 8), so
    every core sees the same per-block degree schedule ghat[l] (compile-time
    constant -> identical SPMD program; all per-core variation is in data).
  * Edge slots are dst-major: slot (block l, k, partition p) holds the k-th
    in-edge of dst p in block l.  A dma_gather pulls the k-th edge row of all
    128 dsts into one [128, cols] SBUF tile (row lands on its dst partition),
    so attention softmax needs no index math on device and the sum over
    in-edges is PSUM accumulation with a constant identity stationary matrix.
  * Each layer's per-node table row = [features bf16 | a_src f32] is built by
    the owning core and AllGather'd (Shared output scratchpad, only the used
    columns transferred) so gathers are core-local.
  * Layer-1 feature columns are stored head-interleaved ([c*4+h] instead of
    [h*128+c]) so the per-edge attention scaling is one tensor_tensor with
    step-1 innermost APs on both operands; W1/bias1/g1/b1 columns and W2 rows
    are permuted on the host to match.
  * LN gamma/beta of the input LN are folded into W1 / alsfix / bias1 on the
    host (exact).  Device LNs use ACT-engine Square+Identity with per-row
    scale/bias operands; per-block epilogues are batched in groups of 5 to
    amortize DVE op overhead and ACT table swaps.
  * Pad edge slots point to table row 0, whose a_src is forced to -1e9 on
    host data (alsfix), making exp(leaky_relu(...)) == 0 exactly.
"""
import sys

sys.path.insert(0, "/opt/trn_rl_repo")

import numpy as np
import ml_dtypes

from concourse import bass, bacc, tile, mybir
from concourse import bass_utils
from concourse.masks import make_identity

BF16 = ml_dtypes.bfloat16
F32 = mybir.dt.float32
BF = mybir.dt.bfloat16
I16 = mybir.dt.int16
AF = mybir.ActivationFunctionType
OP = mybir.AluOpType

# problem constants
N, E = 20000, 320000
D_IN, HID, D_OUT = 128, 128, 32
H1, H2 = 4, 1
EPS = 1e-5

NCORES = 8
P = 128
NPAD = 20480            # padded node count: 160 blocks of 128
NBLK_G = NPAD // P      # 160 global blocks
NPB = NPAD // NCORES    # 2560 nodes per core
NBLK = NPB // P         # 20 blocks per core
NEG = -1e9

T1COLS = 640            # L1 table row (bf16): 512 feats | 4 f32 a_src | pad
T1USED = 520            # columns actually written / AllGather'd
T2COLS = 256            # L2 table row (bf16): 128 feats | 1 f32 a_src | pad
T2USED = 130
KC1 = 24                # max in-edge slots per L1 gather call
KC2 = 16                # max in-edge slots per L2 gather call
NAG = 4                 # AllGather chunks (overlap with producer phase)
GRP = NBLK // NAG       # blocks per epilogue group / AG chunk
NSWQ = 4                # SWDGE queues for gather overlap

# colconst column layout (f32, each value replicated on all 128 partitions)
CC_G1, CC_B1, CC_BIAS1 = 0, 512, 1024
CC_G2, CC_B2, CC_BIAS2 = 1536, 1664, 1792
CC_BO = 1920
NCC = 1952

# head interleave: new L1 feature col c*H1+h <- old col h*HID+c
_PERM = (np.arange(H1 * HID).reshape(H1, HID).T).reshape(-1)  # new idx -> old idx


def _tid(n):
    """table row id of padded-node id n: single AllGather output is
    rank-major (rank c's shard occupies rows [c*NPB, (c+1)*NPB))."""
    blk = n // P
    c = blk % NCORES
    l = blk // NCORES
    return c * NPB + l * P + n % P


def prepare_inputs(x, edge_index):
    """Host graph preprocessing -> per-core arrays + degree schedule."""
    x = np.asarray(x, dtype=np.float32)
    ei = np.asarray(edge_index)
    src = np.concatenate([ei[0], np.arange(N, dtype=ei.dtype)]).astype(np.int64)
    dst = np.concatenate([ei[1], np.arange(N, dtype=ei.dtype)]).astype(np.int64)

    deg = np.bincount(dst, minlength=N)
    order = np.argsort(deg, kind="stable")        # orig node ids, deg ascending
    newid = np.empty(N, dtype=np.int64)           # orig -> padded id
    newid[order] = np.arange(N) + (NPAD - N)      # pads occupy ids [0, 480)

    degp = np.zeros(NPAD, dtype=np.int64)
    degp[newid] = deg
    gmax = degp.reshape(NBLK_G, P).max(axis=1)
    ghat = gmax.reshape(NBLK, NCORES).max(axis=1)         # per local block idx
    S = int(P * ghat.sum())                                # slots per core

    # CSR of in-edges keyed by new dst id
    nd = newid[dst]
    csr_order = np.argsort(nd, kind="stable")
    nsrc_sorted = newid[src[csr_order]]
    indptr = np.zeros(NPAD + 1, dtype=np.int64)
    np.cumsum(np.bincount(nd, minlength=NPAD), out=indptr[1:])

    tid_of = _tid(np.arange(NPAD))

    goff = np.zeros(NBLK, dtype=np.int64)                  # k-slot offsets
    goff[1:] = np.cumsum(ghat)[:-1]

    idxw = np.zeros((NCORES, P, S // 16), dtype=np.int16)
    x_own = np.zeros((NCORES, NPB, D_IN), dtype=np.float32)
    alsfix = np.zeros((NCORES, NPB, 8), dtype=np.float32)

    inv_new = np.full(NPAD, -1, dtype=np.int64)
    inv_new[newid] = np.arange(N)

    for c in range(NCORES):
        gblk = np.arange(NBLK) * NCORES + c                # global block ids
        nid = (gblk[:, None] * P + np.arange(P)).reshape(-1)   # [NPB] padded id
        ov = inv_new[nid]                                  # orig node or -1
        real = ov >= 0
        x_own[c][real] = x[ov[real]]
        alsfix[c][~real, :] = NEG

        idx_flat = np.zeros(S, dtype=np.int16)             # dummy -> row 0
        for l in range(NBLK):
            d0 = nid[l * P:(l + 1) * P]                    # padded ids of block
            base = goff[l] * P
            for p in range(P):
                d = d0[p]
                s0, s1 = indptr[d], indptr[d + 1]
                ks = np.arange(s1 - s0)
                idx_flat[base + ks * P + p] = tid_of[nsrc_sorted[s0:s1]]
        idxw[c] = np.tile(idx_flat.reshape(S // 16, 16).T, (NCORES, 1))

    return {
        "ghat": [int(g) for g in ghat],
        "S": S,
        "idxw": idxw,
        "x_own": x_own,
        "alsfix": alsfix,
        "newid": newid,
    }


def prepare_weights(W1, att1_s, att1_d, bias1, g1, b1, g_in, b_in,
                    W2, att2_s, att2_d, bias2, g2, b2, Wo, bo,
                    alsfix):
    W1 = np.asarray(W1, np.float64)
    W2 = np.asarray(W2, np.float64)
    a1s = np.asarray(att1_s, np.float64)
    a1d = np.asarray(att1_d, np.float64)

    # fold input-LN gamma/beta into W1:  LN0(x) = z*g_in + b_in  with
    # z = (x-mu)*rstd, so  LN0(x) @ W1 = z @ (diag(g_in) W1) + b_in @ W1.
    W1g = np.asarray(g_in, np.float64)[:, None] * W1           # [128, 512]
    c1 = np.asarray(b_in, np.float64) @ W1                     # [512]
    c1h = c1.reshape(H1, HID)
    # constant shift of attention logits per head (exact: u = als+ald+shift)
    ushift = np.einsum("hc,hc->h", c1h, a1s) + np.einsum("hc,hc->h", c1h, a1d)
    alsfix = alsfix.copy()
    alsfix[:, :, 0:4] += ushift[None, None, :]
    bias1f = np.asarray(bias1, np.float64) + c1                # sum(alpha)=1

    w1ext = np.zeros((D_IN, 520), dtype=BF16)
    w1ext[:, :512] = W1g[:, _PERM]                    # head-interleaved cols
    W1gh = W1g.reshape(D_IN, H1, HID)
    w1ext[:, 512:516] = np.einsum("khc,hc->kh", W1gh, a1s)
    w1ext[:, 516:520] = np.einsum("khc,hc->kh", W1gh, a1d)

    w2e = np.zeros((4 * HID, 130), dtype=np.float64)
    w2e[:, :128] = W2[_PERM, :]                       # rows follow interleave
    w2e[:, 128] = W2[_PERM, :] @ np.asarray(att2_s, np.float64)[0]
    w2e[:, 129] = W2[_PERM, :] @ np.asarray(att2_d, np.float64)[0]
    # pack [512, 130] -> [128, 4, 130] (partition p holds rows p, 128+p, ...)
    w2ext = np.ascontiguousarray(
        w2e.reshape(4, P, 130).transpose(1, 0, 2)).astype(BF16)

    woext = np.asarray(Wo, np.float32).astype(BF16)

    cc = np.zeros(NCC, dtype=np.float32)
    cc[CC_G1:CC_G1 + 512] = np.asarray(g1, np.float64)[_PERM]
    cc[CC_B1:CC_B1 + 512] = np.asarray(b1, np.float64)[_PERM]
    cc[CC_BIAS1:CC_BIAS1 + 512] = bias1f[_PERM]
    cc[CC_G2:CC_G2 + 128] = g2
    cc[CC_B2:CC_B2 + 128] = b2
    cc[CC_BIAS2:CC_BIAS2 + 128] = bias2
    cc[CC_BO:CC_BO + 32] = bo
    colconst = np.tile(cc[None, :], (P, 1))

    return {"w1ext": w1ext, "w2ext": w2ext.reshape(P, 4 * 130),
            "woext": woext, "colconst": colconst, "alsfix": alsfix}


def _bap(ap, dims):
    """AP with explicit free-dim [step, count] pairs (partition dim kept)."""
    return bass.AP(ap.tensor, ap.offset, [ap.ap[0]] + [list(d) for d in dims])


def build_program(ghat, num_devices=NCORES):
    S = int(P * sum(ghat))
    goff = np.zeros(NBLK, dtype=np.int64)
    goff[1:] = np.cumsum(ghat)[:-1]

    nc = bacc.Bacc("TRN2", target_bir_lowering=False, debug=False,
                   num_devices=num_devices, num_swdge_queues=NSWQ)

    x_own = nc.dram_tensor("x_own", [NPB, D_IN], F32, kind="ExternalInput")
    idxw = nc.dram_tensor("idxw", [P, S // 16], I16, kind="ExternalInput")
    alsfix = nc.dram_tensor("alsfix", [NPB, 8], F32, kind="ExternalInput")
    w1ext = nc.dram_tensor("w1ext", [D_IN, 520], BF, kind="ExternalInput")
    w2ext = nc.dram_tensor("w2ext", [P, 4 * 130], BF, kind="ExternalInput")
    woext = nc.dram_tensor("woext", [P, D_OUT], BF, kind="ExternalInput")
    colconst = nc.dram_tensor("colconst", [P, NCC], F32, kind="ExternalInput")
    out = nc.dram_tensor("out", [NPB, D_OUT], F32, kind="ExternalOutput")

    rg = [list(range(num_devices))]
    qrr = [0]

    with tile.TileContext(nc) as tc:
        with (
            tc.tile_pool(name="cst", bufs=1) as cst,
            tc.tile_pool(name="wp", bufs=2) as wp,
            tc.tile_pool(name="gp", bufs=2) as gp,
            tc.tile_pool(name="gp2", bufs=4) as gp2,
            tc.tile_pool(name="ps", bufs=2, space="PSUM") as ps,
            tc.tile_pool(name="pss", bufs=2, space="PSUM") as pss,
            tc.tile_pool(name="dram", bufs=1, space="DRAM") as dram,
        ):
            # ---- constants ----
            ident = cst.tile([P, P], BF)
            make_identity(nc, ident[:])
            w1s = cst.tile([P, 520], BF)
            nc.sync.dma_start(w1s[:], w1ext[:])
            w2s = cst.tile([P, 4, 130], BF)
            nc.sync.dma_start(w2s[:], w2ext[:])
            wos = cst.tile([P, D_OUT], BF)
            nc.sync.dma_start(wos[:], woext[:])
            cc = cst.tile([P, NCC], F32)
            nc.sync.dma_start(cc[:], colconst[:])
            idx_sb = cst.tile([P, S // 16], I16)
            nc.sync.dma_start(idx_sb[:], idxw[:])
            afix = cst.tile([P, NBLK, 8], F32)
            nc.sync.dma_start(
                afix[:], bass.AP(alsfix.ap().tensor, 0,
                                 [[8, P], [8 * P, NBLK], [1, 8]]))
            eps_t = cst.tile([P, 1], F32)
            nc.vector.memset(eps_t[:], EPS)
            ald1 = cst.tile([P, NBLK, H1], F32)
            ald2 = cst.tile([P, NBLK, 1], F32)
            zs = cst.tile([P, NBLK, D_OUT], F32)
            sdens = cst.tile([P, NBLK], F32)
            lnds = cst.tile([P, NBLK], F32)

            ag1_in = dram.tile([NPB, T1COLS], BF)
            ag1_out = dram.tile([NPAD, T1COLS], BF, addr_space="Shared")
            ag2_in = dram.tile([NPB, T2COLS], BF)
            ag2_out = dram.tile([NPAD, T2COLS], BF, addr_space="Shared")

            def transpose_to(dst_bf, src_bf):
                pst = pss.tile([P, P], BF, tag="tp")
                nc.tensor.transpose(out=pst[:], in_=src_bf, identity=ident[:])
                nc.vector.tensor_copy(out=dst_bf, in_=pst[:])

            def group_ln(hcat, ncols, cg, cb, tag):
                """Batched LayerNorm over a [P, GRP, ncols] f32 tile, in
                place: hcat <- LN(hcat) * g + b, then caller applies gelu."""
                sq = wp.tile([P, ncols], F32, tag=f"{tag}_sq")
                ssq = wp.tile([P, GRP], F32, tag=f"{tag}_ssq")
                for j in range(GRP):
                    nc.scalar.activation(sq[:], hcat[:, j, :], AF.Square,
                                         accum_out=ssq[:, j:j + 1])
                msum = wp.tile([P, GRP], F32, tag=f"{tag}_ms")
                nc.vector.tensor_reduce(out=msum[:], in_=hcat[:],
                                        axis=mybir.AxisListType.X, op=OP.add)
                mu = wp.tile([P, GRP], F32, tag=f"{tag}_mu")
                nc.vector.tensor_scalar_mul(out=mu[:], in0=msum[:],
                                            scalar1=1.0 / ncols)
                mu2 = wp.tile([P, GRP], F32, tag=f"{tag}_m2")
                nc.vector.tensor_mul(mu2[:], mu[:], mu[:])
                var = wp.tile([P, GRP], F32, tag=f"{tag}_va")
                nc.vector.scalar_tensor_tensor(
                    out=var[:], in0=ssq[:], scalar=1.0 / ncols, in1=mu2[:],
                    op0=OP.mult, op1=OP.subtract)
                sd = wp.tile([P, GRP], F32, tag=f"{tag}_sd")
                nc.scalar.activation(sd[:], var[:], AF.Sqrt, bias=eps_t[:])
                rstd = wp.tile([P, GRP], F32, tag=f"{tag}_rs")
                nc.vector.reciprocal(rstd[:], sd[:])
                nmr = wp.tile([P, GRP], F32, tag=f"{tag}_nm")
                nc.vector.scalar_tensor_tensor(
                    out=nmr[:], in0=mu[:], scalar=-1.0, in1=rstd[:],
                    op0=OP.mult, op1=OP.mult)
                for j in range(GRP):
                    nc.scalar.activation(hcat[:, j, :], hcat[:, j, :],
                                         AF.Identity, bias=nmr[:, j:j + 1],
                                         scale=rstd[:, j:j + 1])
                nc.vector.tensor_tensor(
                    out=hcat[:], in0=hcat[:],
                    in1=_bap(cg, [(0, GRP), (1, ncols)]), op=OP.mult)
                nc.vector.tensor_tensor(
                    out=hcat[:], in0=hcat[:],
                    in1=_bap(cb, [(0, GRP), (1, ncols)]), op=OP.add)

            # ---- phase 0: LN0 (folded gamma/beta) + W1 matmul ----
            for gi in range(NAG):
                xt = wp.tile([P, GRP, D_IN], F32, tag="xt")
                nc.sync.dma_start(
                    xt[:], bass.AP(x_own.ap().tensor,
                                   gi * GRP * P * D_IN,
                                   [[D_IN, P], [P * D_IN, GRP], [1, D_IN]]))
                sq0 = wp.tile([P, D_IN], F32, tag="sq0")
                ssq0 = wp.tile([P, GRP], F32, tag="ssq0")
                for j in range(GRP):
                    nc.scalar.activation(sq0[:], xt[:, j, :], AF.Square,
                                         accum_out=ssq0[:, j:j + 1])
                ms0 = wp.tile([P, GRP], F32, tag="ms0")
                nc.vector.tensor_reduce(out=ms0[:], in_=xt[:],
                                        axis=mybir.AxisListType.X, op=OP.add)
                mu0 = wp.tile([P, GRP], F32, tag="mu0")
                nc.vector.tensor_scalar_mul(out=mu0[:], in0=ms0[:],
                                            scalar1=1.0 / D_IN)
                mu20 = wp.tile([P, GRP], F32, tag="mu20")
                nc.vector.tensor_mul(mu20[:], mu0[:], mu0[:])
                var0 = wp.tile([P, GRP], F32, tag="var0")
                nc.vector.scalar_tensor_tensor(
                    out=var0[:], in0=ssq0[:], scalar=1.0 / D_IN, in1=mu20[:],
                    op0=OP.mult, op1=OP.subtract)
                sd0 = wp.tile([P, GRP], F32, tag="sd0")
                nc.scalar.activation(sd0[:], var0[:], AF.Sqrt, bias=eps_t[:])
                rs0 = wp.tile([P, GRP], F32, tag="rs0")
                nc.vector.reciprocal(rs0[:], sd0[:])
                nm0 = wp.tile([P, GRP], F32, tag="nm0")
                nc.vector.scalar_tensor_tensor(
                    out=nm0[:], in0=mu0[:], scalar=-1.0, in1=rs0[:],
                    op0=OP.mult, op1=OP.mult)
                xnb = wp.tile([P, GRP, D_IN], BF, tag="xnb")
                for j in range(GRP):
                    nc.scalar.activation(xnb[:, j, :], xt[:, j, :],
                                         AF.Identity, bias=nm0[:, j:j + 1],
                                         scale=rs0[:, j:j + 1])
                for j in range(GRP):
                    t = gi * GRP + j
                    xT = wp.tile([P, P], BF, tag="xT")
                    transpose_to(xT[:], xnb[:, j, :])
                    ps1 = ps.tile([P, 512], F32, tag="big")
                    nc.tensor.matmul(ps1[:], lhsT=xT[:], rhs=w1s[:, 0:512],
                                     start=True, stop=True)
                    ps2_t = pss.tile([P, 130], F32, tag="mm2")
                    ps2 = ps2_t[:, 0:8]
                    nc.tensor.matmul(ps2[:], lhsT=xT[:], rhs=w1s[:, 512:520],
                                     start=True, stop=True)
                    tt = wp.tile([P, T1USED], BF, tag="tt")
                    nc.vector.tensor_copy(out=tt[:, 0:512], in_=ps1[:])
                    nc.vector.tensor_tensor(
                        out=tt[:, 512:520].bitcast(F32), in0=ps2[:, 0:4],
                        in1=afix[:, t, 0:4], op=OP.add)
                    nc.vector.tensor_copy(out=ald1[:, t, :], in_=ps2[:, 4:8])
                    nc.sync.dma_start(ag1_in[t * P:(t + 1) * P, 0:T1USED],
                                      tt[:])

            # ---- AllGather the L1 table (Shared output; collectives need
            # contiguous APs so the pad columns ride along)
            nc.gpsimd.collective_compute(
                "AllGather", OP.bypass, replica_groups=rg,
                ins=[ag1_in[:].opt()],
                outs=[ag1_out[:].opt()])

            # ---- phase 2: GAT layer 1 + epilogue + W2 matmul ----
            for gi in range(NAG):
                h1cat = wp.tile([P, GRP, 512], F32, tag="h1cat")
                for j in range(GRP):
                    l = gi * GRP + j
                    g = ghat[l]
                    psA = ps.tile([P, 512], F32, tag="big")
                    den = wp.tile([P, H1], F32, tag="den1")
                    k0 = 0
                    while k0 < g:
                        kn = min(KC1, g - k0)
                        gt = gp.tile([P, KC1, T1COLS], BF, tag="g1")
                        nc.gpsimd.dma_gather(
                            gt[:, 0:kn, :], ag1_out[:],
                            idx_sb[:, 8 * (int(goff[l]) + k0):
                                   8 * (int(goff[l]) + k0 + kn)],
                            P * kn, P * kn, T1COLS, single_packet=False,
                            queue_num=qrr[0] % NSWQ)
                        qrr[0] += 1
                        als_v = gt[:, 0:kn, 512:520].bitcast(F32)
                        u = wp.tile([P, KC1, H1], F32, tag="u1")
                        nc.vector.tensor_tensor(
                            out=u[:, 0:kn, :], in0=als_v,
                            in1=_bap(ald1[:, l, :], [(0, kn), (1, H1)]),
                            op=OP.add)
                        nc.vector.scalar_tensor_tensor(
                            out=u[:, 0:kn, :], in0=u[:, 0:kn, :], scalar=0.2,
                            in1=u[:, 0:kn, :], op0=OP.mult, op1=OP.max)
                        exf = wp.tile([P, KC1, H1], F32, tag="ex1")
                        nc.scalar.activation(exf[:, 0:kn, :], u[:, 0:kn, :],
                                             AF.Exp)
                        dt_ = wp.tile([P, H1], F32, tag="dt1")
                        red = dt_ if k0 else den
                        nc.vector.tensor_reduce(
                            out=red[:], in_=_bap(exf[:], [(1, H1), (H1, kn)]),
                            axis=mybir.AxisListType.X, op=OP.add)
                        if k0:
                            nc.vector.tensor_add(den[:], den[:], dt_[:])
                        exb = wp.tile([P, KC1, H1], BF, tag="exb1")
                        nc.vector.tensor_copy(out=exb[:, 0:kn, :],
                                              in_=exf[:, 0:kn, :])
                        # in-place scale of the gathered features; columns are
                        # head-interleaved so all steps are +/-1 or outer
                        nc.vector.tensor_tensor(
                            out=_bap(gt[:], [(T1COLS, kn), (H1, HID),
                                             (1, H1)]),
                            in0=_bap(gt[:], [(T1COLS, kn), (H1, HID),
                                             (1, H1)]),
                            in1=_bap(exb[:], [(H1, kn), (0, HID), (1, H1)]),
                            op=OP.mult)
                        for k in range(kn):
                            nc.tensor.matmul(psA[:], lhsT=ident[:],
                                             rhs=gt[:, k, 0:512],
                                             start=(k0 + k == 0),
                                             stop=(k0 + k == g - 1))
                        k0 += kn
                    nc.vector.tensor_scalar_add(out=den[:], in0=den[:],
                                                scalar1=1e-30)
                    denr = wp.tile([P, H1], F32, tag="dr1")
                    nc.vector.reciprocal(denr[:], den[:])
                    # h1 = psA * denr(per head, interleaved bcast) + bias1
                    nc.vector.tensor_tensor(
                        out=_bap(h1cat[:, j, :], [(H1, HID), (1, H1)]),
                        in0=_bap(psA[:], [(H1, HID), (1, H1)]),
                        in1=_bap(denr[:], [(0, HID), (1, H1)]),
                        op=OP.mult)
                    nc.vector.tensor_tensor(
                        out=h1cat[:, j, :], in0=h1cat[:, j, :],
                        in1=cc[:, CC_BIAS1:CC_BIAS1 + 512], op=OP.add)

                # ---- group epilogue: LN1 + gelu (batched over GRP blocks)
                group_ln(h1cat, 512, cc[:, CC_G1:CC_G1 + 512],
                         cc[:, CC_B1:CC_B1 + 512], "ln1")
                h1b = wp.tile([P, GRP, 512], BF, tag="h1b")
                nc.scalar.activation(h1b[:], h1cat[:], AF.Gelu)
                for j in range(GRP):
                    l = gi * GRP + j
                    ps3 = pss.tile([P, 130], F32, tag="mm2")
                    for cch in range(4):
                        hT = wp.tile([P, P], BF, tag="hT")
                        transpose_to(hT[:], h1b[:, j, cch * P:(cch + 1) * P])
                        nc.tensor.matmul(ps3[:], lhsT=hT[:], rhs=w2s[:, cch, :],
                                         start=(cch == 0), stop=(cch == 3))
                    t2 = wp.tile([P, T2USED], BF, tag="t2")
                    nc.vector.tensor_copy(out=t2[:, 0:128], in_=ps3[:, 0:128])
                    nc.vector.tensor_tensor(
                        out=t2[:, 128:130].bitcast(F32), in0=ps3[:, 128:129],
                        in1=afix[:, l, 4:5], op=OP.add)
                    nc.vector.tensor_copy(out=ald2[:, l, :],
                                          in_=ps3[:, 129:130])
                    nc.sync.dma_start(ag2_in[l * P:(l + 1) * P, 0:T2USED],
                                      t2[:])

            # ---- AllGather the L2 table (Shared output)
            nc.gpsimd.collective_compute(
                "AllGather", OP.bypass, replica_groups=rg,
                ins=[ag2_in[:].opt()],
                outs=[ag2_out[:].opt()])

            # ---- phase 4: GAT layer 2 + epilogue + output head ----
            for gi in range(NAG):
                h2cat = wp.tile([P, GRP, 128], F32, tag="h2cat")
                for j in range(GRP):
                    l = gi * GRP + j
                    g = ghat[l]
                    psB_t = ps.tile([P, 512], F32, tag="big")
                    psB = psB_t[:, 0:128]
                    den = wp.tile([P, 1], F32, tag="den2")
                    k0 = 0
                    while k0 < g:
                        kn = min(KC2, g - k0)
                        gt = gp2.tile([P, KC2, T2COLS], BF, tag="g2")
                        nc.gpsimd.dma_gather(
                            gt[:, 0:kn, :], ag2_out[:],
                            idx_sb[:, 8 * (int(goff[l]) + k0):
                                   8 * (int(goff[l]) + k0 + kn)],
                            P * kn, P * kn, T2COLS, single_packet=False,
                            queue_num=qrr[0] % NSWQ)
                        qrr[0] += 1
                        als_v = gt[:, 0:kn, 128:130].bitcast(F32)
                        u = wp.tile([P, KC2, 1], F32, tag="u2")
                        nc.vector.tensor_tensor(
                            out=u[:, 0:kn, :], in0=als_v,
                            in1=_bap(ald2[:, l, :], [(0, kn), (1, 1)]),
                            op=OP.add)
                        nc.vector.scalar_tensor_tensor(
                            out=u[:, 0:kn, :], in0=u[:, 0:kn, :], scalar=0.2,
                            in1=u[:, 0:kn, :], op0=OP.mult, op1=OP.max)
                        exf = wp.tile([P, KC2, 1], F32, tag="ex2")
                        dt_ = wp.tile([P, 1], F32, tag="dt2")
                        red = dt_ if k0 else den
                        nc.scalar.activation(exf[:, 0:kn, :], u[:, 0:kn, :],
                                             AF.Exp, accum_out=red[:])
                        if k0:
                            nc.vector.tensor_add(den[:], den[:], dt_[:])
                        exb = wp.tile([P, KC2, 1], BF, tag="exb2")
                        nc.vector.tensor_copy(out=exb[:, 0:kn, :],
                                              in_=exf[:, 0:kn, :])
                        nc.vector.tensor_tensor(
                            out=_bap(gt[:], [(T2COLS, kn), (1, 128)]),
                            in0=_bap(gt[:], [(T2COLS, kn), (1, 128)]),
                            in1=_bap(exb[:], [(1, kn), (0, 128)]),
                            op=OP.mult)
                        for k in range(kn):
                            nc.tensor.matmul(psB[:], lhsT=ident[:],
                                             rhs=gt[:, k, 0:128],
                                             start=(k0 + k == 0),
                                             stop=(k0 + k == g - 1))
                        k0 += kn
                    nc.vector.tensor_scalar_add(out=den[:], in0=den[:],
                                                scalar1=1e-30)
                    denr = wp.tile([P, 1], F32, tag="dr2")
                    nc.vector.reciprocal(denr[:], den[:])
                    nc.vector.scalar_tensor_tensor(
                        out=h2cat[:, j, :], in0=psB[:], scalar=denr[:],
                        in1=cc[:, CC_BIAS2:CC_BIAS2 + 128],
                        op0=OP.mult, op1=OP.add)

                # ---- group epilogue: LN2 + gelu + output head
                group_ln(h2cat, 128, cc[:, CC_G2:CC_G2 + 128],
                         cc[:, CC_B2:CC_B2 + 128], "ln2")
                h2b = wp.tile([P, GRP, 128], BF, tag="h2b")
                nc.scalar.activation(h2b[:], h2cat[:], AF.Gelu)
                for j in range(GRP):
                    l = gi * GRP + j
                    hoT = wp.tile([P, P], BF, tag="hoT")
                    transpose_to(hoT[:], h2b[:, j, :])
                    pso_t = pss.tile([P, 130], F32, tag="mm2")
                    pso = pso_t[:, 0:D_OUT]
                    nc.tensor.matmul(pso[:], lhsT=hoT[:], rhs=wos[:],
                                     start=True, stop=True)
                    z = wp.tile([P, D_OUT], F32, tag="z")
                    nc.vector.tensor_tensor(out=z[:], in0=pso[:],
                                            in1=cc[:, CC_BO:CC_BO + 32],
                                            op=OP.add)
                    m = wp.tile([P, 1], F32, tag="zm")
                    nc.vector.tensor_reduce(out=m[:], in_=z[:],
                                            axis=mybir.AxisListType.X,
                                            op=OP.max)
                    nc.vector.tensor_scalar_sub(out=zs[:, l, :], in0=z[:],
                                                scalar1=m[:])

            # ---- log-softmax tail (batched: one Ln table load) ----
            ez = wp.tile([P, D_OUT], F32, tag="ez")
            for l in range(NBLK):
                nc.scalar.activation(ez[:], zs[:, l, :], AF.Exp,
                                     accum_out=sdens[:, l:l + 1])
            nc.scalar.activation(lnds[:], sdens[:], AF.Ln)
            for gi in range(NAG):
                res = wp.tile([P, GRP, D_OUT], F32, tag="res")
                for j in range(GRP):
                    l = gi * GRP + j
                    nc.vector.tensor_scalar_sub(out=res[:, j, :],
                                                in0=zs[:, l, :],
                                                scalar1=lnds[:, l:l + 1])
                nc.sync.dma_start(
                    bass.AP(out.ap().tensor, gi * GRP * P * D_OUT,
                            [[D_OUT, P], [P * D_OUT, GRP], [1, D_OUT]]),
                    res[:])

    nc.compile()
    return nc


_CACHE = {}
_LAST_RUN = {}


def kernel(x, edge_index, g_in, b_in, W1, att1_s, att1_d, bias1, g1, b1,
           W2, att2_s, att2_d, bias2, g2, b2, Wo, bo):
    prep = prepare_inputs(x, edge_index)
    wts = prepare_weights(W1, att1_s, att1_d, bias1, g1, b1, g_in, b_in,
                          W2, att2_s, att2_d, bias2, g2, b2, Wo, bo,
                          prep["alsfix"])

    key = tuple(prep["ghat"])
    if key not in _CACHE:
        _CACHE[key] = build_program(prep["ghat"])
    nc = _CACHE[key]

    in_maps = []
    for c in range(NCORES):
        in_maps.append({
            "x_own": prep["x_own"][c],
            "idxw": prep["idxw"][c],
            "alsfix": wts["alsfix"][c],
            "w1ext": wts["w1ext"],
            "w2ext": wts["w2ext"].astype(BF16),
            "woext": wts["woext"],
            "colconst": wts["colconst"],
        })

    _LAST_RUN.update(nc=nc, in_maps=in_maps, prep=prep)
    res = bass_utils.run_bass_kernel_spmd(nc, in_maps,
                                          core_ids=list(range(NCORES)))
    outs = [res.results[c]["out"] for c in range(NCORES)]

    newid = prep["newid"]
    blk = newid // P
    core = blk % NCORES
    row = (blk // NCORES) * P + newid % P
    full = np.empty((N, D_OUT), dtype=np.float32)
    for c in range(NCORES):
        sel = core == c
        full[sel] = outs[c][row[sel]]
    return full


# revision 27
# speedup vs baseline: 1.1176x; 1.1176x over previous
"""Distributed 2-layer GAT kernel for 8 Trainium2 NeuronCores.

Strategy (host graph preprocessing + device SPMD kernel):
  * Nodes are relabeled by in-degree (ascending) and padded to 20480 ids.
    Blocks of 128 consecutive ids then have near-uniform in-degree, and the
    160 blocks are dealt round-robin to the 8 cores (core = block % 8), so
    every core sees the same per-block degree schedule ghat[l] (compile-time
    constant -> identical SPMD program; all per-core variation is in data).
  * Edge slots are dst-major: slot (block l, k, partition p) holds the k-th
    in-edge of dst p in block l.  A dma_gather pulls the k-th edge row of all
    128 dsts into one [128, cols] SBUF tile (row lands on its dst partition),
    so attention softmax needs no index math on device and the sum over
    in-edges is PSUM accumulation with a constant identity stationary matrix.
  * Each layer's per-node table row = [features bf16 | a_src f32] is built by
    the owning core and AllGather'd (Shared output scratchpad, only the used
    columns transferred) so gathers are core-local.
  * Layer-1 feature columns are stored head-interleaved ([c*4+h] instead of
    [h*128+c]) so the per-edge attention scaling is one tensor_tensor with
    step-1 innermost APs on both operands; W1/bias1/g1/b1 columns and W2 rows
    are permuted on the host to match.
  * LN gamma/beta of the input LN are folded into W1 / alsfix / bias1 on the
    host (exact).  Device LNs use ACT-engine Square+Identity with per-row
    scale/bias operands; per-block epilogues are batched in groups of 5 to
    amortize DVE op overhead and ACT table swaps.
  * Pad edge slots point to table row 0, whose a_src is forced to -1e9 on
    host data (alsfix), making exp(leaky_relu(...)) == 0 exactly.
"""
import sys

sys.path.insert(0, "/opt/trn_rl_repo")

import numpy as np
import ml_dtypes

from concourse import bass, bacc, tile, mybir
from concourse import bass_utils
from concourse.masks import make_identity

BF16 = ml_dtypes.bfloat16
F32 = mybir.dt.float32
BF = mybir.dt.bfloat16
I16 = mybir.dt.int16
AF = mybir.ActivationFunctionType
OP = mybir.AluOpType

# problem constants
N, E = 20000, 320000
D_IN, HID, D_OUT = 128, 128, 32
H1, H2 = 4, 1
EPS = 1e-5

NCORES = 8
P = 128
NPAD = 20480            # padded node count: 160 blocks of 128
NBLK_G = NPAD // P      # 160 global blocks
NPB = NPAD // NCORES    # 2560 nodes per core
NBLK = NPB // P         # 20 blocks per core
NEG = -1e9

T1COLS = 640            # L1 table row (bf16): 512 feats | 4 f32 a_src | pad
T1USED = 520            # columns actually written / AllGather'd
T2COLS = 256            # L2 table row (bf16): 128 feats | 1 f32 a_src | pad
T2USED = 130
KC1 = 16                # max in-edge slots per L1 gather call
KC2 = 16                # max in-edge slots per L2 gather call
NAG = 4                 # AllGather chunks (overlap with producer phase)
GRP = NBLK // NAG       # blocks per epilogue group / AG chunk
NSWQ = 4                # SWDGE queues for gather overlap

# colconst column layout (f32, each value replicated on all 128 partitions)
CC_G1, CC_B1, CC_BIAS1 = 0, 512, 1024
CC_G2, CC_B2, CC_BIAS2 = 1536, 1664, 1792
CC_BO = 1920
NCC = 1952

# head interleave: new L1 feature col c*H1+h <- old col h*HID+c
_PERM = (np.arange(H1 * HID).reshape(H1, HID).T).reshape(-1)  # new idx -> old idx


def _tid(n):
    """table row id of padded-node id n: single AllGather output is
    rank-major (rank c's shard occupies rows [c*NPB, (c+1)*NPB))."""
    blk = n // P
    c = blk % NCORES
    l = blk // NCORES
    return c * NPB + l * P + n % P


def prepare_inputs(x, edge_index):
    """Host graph preprocessing -> per-core arrays + degree schedule."""
    x = np.asarray(x, dtype=np.float32)
    ei = np.asarray(edge_index)
    src = np.concatenate([ei[0], np.arange(N, dtype=ei.dtype)]).astype(np.int64)
    dst = np.concatenate([ei[1], np.arange(N, dtype=ei.dtype)]).astype(np.int64)

    deg = np.bincount(dst, minlength=N)
    order = np.argsort(deg, kind="stable")        # orig node ids, deg ascending
    newid = np.empty(N, dtype=np.int64)           # orig -> padded id
    newid[order] = np.arange(N) + (NPAD - N)      # pads occupy ids [0, 480)

    degp = np.zeros(NPAD, dtype=np.int64)
    degp[newid] = deg
    gmax = degp.reshape(NBLK_G, P).max(axis=1)
    ghat = gmax.reshape(NBLK, NCORES).max(axis=1)         # per local block idx
    S = int(P * ghat.sum())                                # slots per core

    # CSR of in-edges keyed by new dst id
    nd = newid[dst]
    csr_order = np.argsort(nd, kind="stable")
    nsrc_sorted = newid[src[csr_order]]
    indptr = np.zeros(NPAD + 1, dtype=np.int64)
    np.cumsum(np.bincount(nd, minlength=NPAD), out=indptr[1:])

    tid_of = _tid(np.arange(NPAD))

    goff = np.zeros(NBLK, dtype=np.int64)                  # k-slot offsets
    goff[1:] = np.cumsum(ghat)[:-1]

    idxw = np.zeros((NCORES, P, S // 16), dtype=np.int16)
    x_own = np.zeros((NCORES, NPB, D_IN), dtype=np.float32)
    alsfix = np.zeros((NCORES, NPB, 8), dtype=np.float32)

    inv_new = np.full(NPAD, -1, dtype=np.int64)
    inv_new[newid] = np.arange(N)

    for c in range(NCORES):
        gblk = np.arange(NBLK) * NCORES + c                # global block ids
        nid = (gblk[:, None] * P + np.arange(P)).reshape(-1)   # [NPB] padded id
        ov = inv_new[nid]                                  # orig node or -1
        real = ov >= 0
        x_own[c][real] = x[ov[real]]
        alsfix[c][~real, :] = NEG

        idx_flat = np.zeros(S, dtype=np.int16)             # dummy -> row 0
        for l in range(NBLK):
            d0 = nid[l * P:(l + 1) * P]                    # padded ids of block
            base = goff[l] * P
            for p in range(P):
                d = d0[p]
                s0, s1 = indptr[d], indptr[d + 1]
                ks = np.arange(s1 - s0)
                idx_flat[base + ks * P + p] = tid_of[nsrc_sorted[s0:s1]]
        idxw[c] = np.tile(idx_flat.reshape(S // 16, 16).T, (NCORES, 1))

    return {
        "ghat": [int(g) for g in ghat],
        "S": S,
        "idxw": idxw,
        "x_own": x_own,
        "alsfix": alsfix,
        "newid": newid,
    }


def prepare_weights(W1, att1_s, att1_d, bias1, g1, b1, g_in, b_in,
                    W2, att2_s, att2_d, bias2, g2, b2, Wo, bo,
                    alsfix):
    W1 = np.asarray(W1, np.float64)
    W2 = np.asarray(W2, np.float64)
    a1s = np.asarray(att1_s, np.float64)
    a1d = np.asarray(att1_d, np.float64)

    # fold input-LN gamma/beta into W1:  LN0(x) = z*g_in + b_in  with
    # z = (x-mu)*rstd, so  LN0(x) @ W1 = z @ (diag(g_in) W1) + b_in @ W1.
    W1g = np.asarray(g_in, np.float64)[:, None] * W1           # [128, 512]
    c1 = np.asarray(b_in, np.float64) @ W1                     # [512]
    c1h = c1.reshape(H1, HID)
    # constant shift of attention logits per head (exact: u = als+ald+shift)
    ushift = np.einsum("hc,hc->h", c1h, a1s) + np.einsum("hc,hc->h", c1h, a1d)
    alsfix = alsfix.copy()
    alsfix[:, :, 0:4] += ushift[None, None, :]
    bias1f = np.asarray(bias1, np.float64) + c1                # sum(alpha)=1

    w1ext = np.zeros((D_IN, 520), dtype=BF16)
    w1ext[:, :512] = W1g[:, _PERM]                    # head-interleaved cols
    W1gh = W1g.reshape(D_IN, H1, HID)
    w1ext[:, 512:516] = np.einsum("khc,hc->kh", W1gh, a1s)
    w1ext[:, 516:520] = np.einsum("khc,hc->kh", W1gh, a1d)

    w2e = np.zeros((4 * HID, 130), dtype=np.float64)
    w2e[:, :128] = W2[_PERM, :]                       # rows follow interleave
    w2e[:, 128] = W2[_PERM, :] @ np.asarray(att2_s, np.float64)[0]
    w2e[:, 129] = W2[_PERM, :] @ np.asarray(att2_d, np.float64)[0]
    # pack [512, 130] -> [128, 4, 130] (partition p holds rows p, 128+p, ...)
    w2ext = np.ascontiguousarray(
        w2e.reshape(4, P, 130).transpose(1, 0, 2)).astype(BF16)

    woext = np.asarray(Wo, np.float32).astype(BF16)

    cc = np.zeros(NCC, dtype=np.float32)
    cc[CC_G1:CC_G1 + 512] = np.asarray(g1, np.float64)[_PERM]
    cc[CC_B1:CC_B1 + 512] = np.asarray(b1, np.float64)[_PERM]
    cc[CC_BIAS1:CC_BIAS1 + 512] = bias1f[_PERM]
    cc[CC_G2:CC_G2 + 128] = g2
    cc[CC_B2:CC_B2 + 128] = b2
    cc[CC_BIAS2:CC_BIAS2 + 128] = bias2
    cc[CC_BO:CC_BO + 32] = bo
    colconst = np.tile(cc[None, :], (P, 1))

    return {"w1ext": w1ext, "w2ext": w2ext.reshape(P, 4 * 130),
            "woext": woext, "colconst": colconst, "alsfix": alsfix}


def _bap(ap, dims):
    """AP with explicit free-dim [step, count] pairs (partition dim kept)."""
    return bass.AP(ap.tensor, ap.offset, [ap.ap[0]] + [list(d) for d in dims])


def build_program(ghat, num_devices=NCORES):
    S = int(P * sum(ghat))
    goff = np.zeros(NBLK, dtype=np.int64)
    goff[1:] = np.cumsum(ghat)[:-1]

    nc = bacc.Bacc("TRN2", target_bir_lowering=False, debug=False,
                   num_devices=num_devices, num_swdge_queues=NSWQ)

    x_own = nc.dram_tensor("x_own", [NPB, D_IN], F32, kind="ExternalInput")
    idxw = nc.dram_tensor("idxw", [P, S // 16], I16, kind="ExternalInput")
    alsfix = nc.dram_tensor("alsfix", [NPB, 8], F32, kind="ExternalInput")
    w1ext = nc.dram_tensor("w1ext", [D_IN, 520], BF, kind="ExternalInput")
    w2ext = nc.dram_tensor("w2ext", [P, 4 * 130], BF, kind="ExternalInput")
    woext = nc.dram_tensor("woext", [P, D_OUT], BF, kind="ExternalInput")
    colconst = nc.dram_tensor("colconst", [P, NCC], F32, kind="ExternalInput")
    out = nc.dram_tensor("out", [NPB, D_OUT], F32, kind="ExternalOutput")

    rg = [list(range(num_devices))]
    qrr = [0]

    with tile.TileContext(nc) as tc:
        with (
            tc.tile_pool(name="cst", bufs=1) as cst,
            tc.tile_pool(name="wp", bufs=2) as wp,
            tc.tile_pool(name="gp", bufs=3) as gp,
            tc.tile_pool(name="gp2", bufs=4) as gp2,
            tc.tile_pool(name="ps", bufs=2, space="PSUM") as ps,
            tc.tile_pool(name="pss", bufs=2, space="PSUM") as pss,
            tc.tile_pool(name="dram", bufs=1, space="DRAM") as dram,
        ):
            # ---- constants ----
            ident = cst.tile([P, P], BF)
            make_identity(nc, ident[:])
            w1s = cst.tile([P, 520], BF)
            nc.sync.dma_start(w1s[:], w1ext[:])
            w2s = cst.tile([P, 4, 130], BF)
            nc.sync.dma_start(w2s[:], w2ext[:])
            wos = cst.tile([P, D_OUT], BF)
            nc.sync.dma_start(wos[:], woext[:])
            cc = cst.tile([P, NCC], F32)
            nc.sync.dma_start(cc[:], colconst[:])
            idx_sb = cst.tile([P, S // 16], I16)
            nc.sync.dma_start(idx_sb[:], idxw[:])
            afix = cst.tile([P, NBLK, 8], F32)
            nc.sync.dma_start(
                afix[:], bass.AP(alsfix.ap().tensor, 0,
                                 [[8, P], [8 * P, NBLK], [1, 8]]))
            eps_t = cst.tile([P, 1], F32)
            nc.vector.memset(eps_t[:], EPS)
            ald1 = cst.tile([P, NBLK, H1], F32)
            ald2 = cst.tile([P, NBLK, 1], F32)
            zs = cst.tile([P, NBLK, D_OUT], F32)
            sdens = cst.tile([P, NBLK], F32)
            lnds = cst.tile([P, NBLK], F32)

            ag1_in = dram.tile([NPB, T1COLS], BF)
            ag1_out = dram.tile([NPAD, T1COLS], BF, addr_space="Shared")
            ag2_in = dram.tile([NPB, T2COLS], BF)
            ag2_out = dram.tile([NPAD, T2COLS], BF, addr_space="Shared")

            def transpose_to(dst_bf, src_bf):
                pst = pss.tile([P, P], BF, tag="tp")
                nc.tensor.transpose(out=pst[:], in_=src_bf, identity=ident[:])
                nc.vector.tensor_copy(out=dst_bf, in_=pst[:])

            def group_ln(hcat, ncols, cg, cb, tag):
                """Batched LayerNorm over a [P, GRP, ncols] f32 tile, in
                place: hcat <- LN(hcat) * g + b, then caller applies gelu."""
                sq = wp.tile([P, ncols], F32, tag=f"{tag}_sq")
                ssq = wp.tile([P, GRP], F32, tag=f"{tag}_ssq")
                for j in range(GRP):
                    nc.scalar.activation(sq[:], hcat[:, j, :], AF.Square,
                                         accum_out=ssq[:, j:j + 1])
                msum = wp.tile([P, GRP], F32, tag=f"{tag}_ms")
                nc.vector.tensor_reduce(out=msum[:], in_=hcat[:],
                                        axis=mybir.AxisListType.X, op=OP.add)
                mu = wp.tile([P, GRP], F32, tag=f"{tag}_mu")
                nc.vector.tensor_scalar_mul(out=mu[:], in0=msum[:],
                                            scalar1=1.0 / ncols)
                mu2 = wp.tile([P, GRP], F32, tag=f"{tag}_m2")
                nc.vector.tensor_mul(mu2[:], mu[:], mu[:])
                var = wp.tile([P, GRP], F32, tag=f"{tag}_va")
                nc.vector.scalar_tensor_tensor(
                    out=var[:], in0=ssq[:], scalar=1.0 / ncols, in1=mu2[:],
                    op0=OP.mult, op1=OP.subtract)
                sd = wp.tile([P, GRP], F32, tag=f"{tag}_sd")
                nc.scalar.activation(sd[:], var[:], AF.Sqrt, bias=eps_t[:])
                rstd = wp.tile([P, GRP], F32, tag=f"{tag}_rs")
                nc.vector.reciprocal(rstd[:], sd[:])
                nmr = wp.tile([P, GRP], F32, tag=f"{tag}_nm")
                nc.vector.scalar_tensor_tensor(
                    out=nmr[:], in0=mu[:], scalar=-1.0, in1=rstd[:],
                    op0=OP.mult, op1=OP.mult)
                for j in range(GRP):
                    nc.scalar.activation(hcat[:, j, :], hcat[:, j, :],
                                         AF.Identity, bias=nmr[:, j:j + 1],
                                         scale=rstd[:, j:j + 1])
                nc.vector.tensor_tensor(
                    out=hcat[:], in0=hcat[:],
                    in1=_bap(cg, [(0, GRP), (1, ncols)]), op=OP.mult)
                nc.vector.tensor_tensor(
                    out=hcat[:], in0=hcat[:],
                    in1=_bap(cb, [(0, GRP), (1, ncols)]), op=OP.add)

            # ---- phase 0: LN0 (folded gamma/beta) + W1 matmul ----
            for gi in range(NAG):
                xt = wp.tile([P, GRP, D_IN], F32, tag="xt")
                nc.sync.dma_start(
                    xt[:], bass.AP(x_own.ap().tensor,
                                   gi * GRP * P * D_IN,
                                   [[D_IN, P], [P * D_IN, GRP], [1, D_IN]]))
                sq0 = wp.tile([P, GRP, D_IN], F32, tag="sq0")
                nc.vector.tensor_mul(sq0[:], xt[:], xt[:])
                ssq0 = wp.tile([P, GRP], F32, tag="ssq0")
                nc.vector.tensor_reduce(out=ssq0[:], in_=sq0[:],
                                        axis=mybir.AxisListType.X, op=OP.add)
                ms0 = wp.tile([P, GRP], F32, tag="ms0")
                nc.vector.tensor_reduce(out=ms0[:], in_=xt[:],
                                        axis=mybir.AxisListType.X, op=OP.add)
                mu0 = wp.tile([P, GRP], F32, tag="mu0")
                nc.vector.tensor_scalar_mul(out=mu0[:], in0=ms0[:],
                                            scalar1=1.0 / D_IN)
                mu20 = wp.tile([P, GRP], F32, tag="mu20")
                nc.vector.tensor_mul(mu20[:], mu0[:], mu0[:])
                var0 = wp.tile([P, GRP], F32, tag="var0")
                nc.vector.scalar_tensor_tensor(
                    out=var0[:], in0=ssq0[:], scalar=1.0 / D_IN, in1=mu20[:],
                    op0=OP.mult, op1=OP.subtract)
                sd0 = wp.tile([P, GRP], F32, tag="sd0")
                nc.scalar.activation(sd0[:], var0[:], AF.Sqrt, bias=eps_t[:])
                rs0 = wp.tile([P, GRP], F32, tag="rs0")
                nc.vector.reciprocal(rs0[:], sd0[:])
                xc0 = wp.tile([P, GRP, D_IN], F32, tag="xc0")
                nc.vector.tensor_tensor(
                    out=xc0[:], in0=xt[:],
                    in1=_bap(mu0[:], [(1, GRP), (0, D_IN)]), op=OP.subtract)
                xnb = wp.tile([P, GRP, D_IN], BF, tag="xnb")
                nc.vector.tensor_tensor(
                    out=xnb[:], in0=xc0[:],
                    in1=_bap(rs0[:], [(1, GRP), (0, D_IN)]), op=OP.mult)
                for j in range(GRP):
                    t = gi * GRP + j
                    xT = wp.tile([P, P], BF, tag="xT")
                    transpose_to(xT[:], xnb[:, j, :])
                    ps1 = ps.tile([P, 512], F32, tag="big")
                    nc.tensor.matmul(ps1[:], lhsT=xT[:], rhs=w1s[:, 0:512],
                                     start=True, stop=True)
                    ps2_t = pss.tile([P, 130], F32, tag="mm2")
                    ps2 = ps2_t[:, 0:8]
                    nc.tensor.matmul(ps2[:], lhsT=xT[:], rhs=w1s[:, 512:520],
                                     start=True, stop=True)
                    tt = wp.tile([P, T1USED], BF, tag="tt")
                    nc.vector.tensor_copy(out=tt[:, 0:512], in_=ps1[:])
                    nc.vector.tensor_tensor(
                        out=tt[:, 512:520].bitcast(F32), in0=ps2[:, 0:4],
                        in1=afix[:, t, 0:4], op=OP.add)
                    nc.vector.tensor_copy(out=ald1[:, t, :], in_=ps2[:, 4:8])
                    nc.sync.dma_start(ag1_in[t * P:(t + 1) * P, 0:T1USED],
                                      tt[:])

            # ---- AllGather the L1 table (Shared output; collectives need
            # contiguous APs so the pad columns ride along)
            nc.gpsimd.collective_compute(
                "AllGather", OP.bypass, replica_groups=rg,
                ins=[ag1_in[:].opt()],
                outs=[ag1_out[:].opt()])

            # ---- phase 2: GAT layer 1 + epilogue + W2 matmul ----
            for gi in range(NAG):
                h1cat = wp.tile([P, GRP, 512], F32, tag="h1cat")
                for j in range(GRP):
                    l = gi * GRP + j
                    g = ghat[l]
                    psA = ps.tile([P, 512], F32, tag="big")
                    den = wp.tile([P, H1], F32, tag="den1")
                    k0 = 0
                    while k0 < g:
                        kn = min(KC1, g - k0)
                        gt = gp.tile([P, KC1, T1COLS], BF, tag="g1")
                        nc.gpsimd.dma_gather(
                            gt[:, 0:kn, :], ag1_out[:],
                            idx_sb[:, 8 * (int(goff[l]) + k0):
                                   8 * (int(goff[l]) + k0 + kn)],
                            P * kn, P * kn, T1COLS, single_packet=False,
                            queue_num=qrr[0] % NSWQ)
                        qrr[0] += 1
                        als_v = gt[:, 0:kn, 512:520].bitcast(F32)
                        u = wp.tile([P, KC1, H1], F32, tag="u1")
                        nc.vector.tensor_tensor(
                            out=u[:, 0:kn, :], in0=als_v,
                            in1=_bap(ald1[:, l, :], [(0, kn), (1, H1)]),
                            op=OP.add)
                        nc.vector.scalar_tensor_tensor(
                            out=u[:, 0:kn, :], in0=u[:, 0:kn, :], scalar=0.2,
                            in1=u[:, 0:kn, :], op0=OP.mult, op1=OP.max)
                        exf = wp.tile([P, KC1, H1], F32, tag="ex1")
                        nc.scalar.activation(exf[:, 0:kn, :], u[:, 0:kn, :],
                                             AF.Exp)
                        dt_ = wp.tile([P, H1], F32, tag="dt1")
                        red = dt_ if k0 else den
                        nc.vector.tensor_reduce(
                            out=red[:], in_=_bap(exf[:], [(1, H1), (H1, kn)]),
                            axis=mybir.AxisListType.X, op=OP.add)
                        if k0:
                            nc.vector.tensor_add(den[:], den[:], dt_[:])
                        exb = wp.tile([P, KC1, H1], BF, tag="exb1")
                        nc.vector.tensor_copy(out=exb[:, 0:kn, :],
                                              in_=exf[:, 0:kn, :])
                        # in-place scale of the gathered features; columns are
                        # head-interleaved so all steps are +/-1 or outer
                        nc.vector.tensor_tensor(
                            out=_bap(gt[:], [(T1COLS, kn), (H1, HID),
                                             (1, H1)]),
                            in0=_bap(gt[:], [(T1COLS, kn), (H1, HID),
                                             (1, H1)]),
                            in1=_bap(exb[:], [(H1, kn), (0, HID), (1, H1)]),
                            op=OP.mult)
                        for k in range(kn):
                            nc.tensor.matmul(psA[:], lhsT=ident[:],
                                             rhs=gt[:, k, 0:512],
                                             start=(k0 + k == 0),
                                             stop=(k0 + k == g - 1))
                        k0 += kn
                    nc.vector.tensor_scalar_add(out=den[:], in0=den[:],
                                                scalar1=1e-30)
                    denr = wp.tile([P, H1], F32, tag="dr1")
                    nc.vector.reciprocal(denr[:], den[:])
                    # h1 = psA * denr(per head, interleaved bcast) + bias1
                    nc.vector.tensor_tensor(
                        out=_bap(h1cat[:, j, :], [(H1, HID), (1, H1)]),
                        in0=_bap(psA[:], [(H1, HID), (1, H1)]),
                        in1=_bap(denr[:], [(0, HID), (1, H1)]),
                        op=OP.mult)
                    nc.vector.tensor_tensor(
                        out=h1cat[:, j, :], in0=h1cat[:, j, :],
                        in1=cc[:, CC_BIAS1:CC_BIAS1 + 512], op=OP.add)

                # ---- group epilogue: LN1 + gelu (batched over GRP blocks)
                group_ln(h1cat, 512, cc[:, CC_G1:CC_G1 + 512],
                         cc[:, CC_B1:CC_B1 + 512], "ln1")
                h1b = wp.tile([P, GRP, 512], BF, tag="h1b")
                nc.scalar.activation(h1b[:], h1cat[:], AF.Gelu)
                for j in range(GRP):
                    l = gi * GRP + j
                    ps3 = pss.tile([P, 130], F32, tag="mm2")
                    for cch in range(4):
                        hT = wp.tile([P, P], BF, tag="hT")
                        transpose_to(hT[:], h1b[:, j, cch * P:(cch + 1) * P])
                        nc.tensor.matmul(ps3[:], lhsT=hT[:], rhs=w2s[:, cch, :],
                                         start=(cch == 0), stop=(cch == 3))
                    t2 = wp.tile([P, T2USED], BF, tag="t2")
                    nc.vector.tensor_copy(out=t2[:, 0:128], in_=ps3[:, 0:128])
                    nc.vector.tensor_tensor(
                        out=t2[:, 128:130].bitcast(F32), in0=ps3[:, 128:129],
                        in1=afix[:, l, 4:5], op=OP.add)
                    nc.vector.tensor_copy(out=ald2[:, l, :],
                                          in_=ps3[:, 129:130])
                    nc.sync.dma_start(ag2_in[l * P:(l + 1) * P, 0:T2USED],
                                      t2[:])

            # ---- AllGather the L2 table (Shared output)
            nc.gpsimd.collective_compute(
                "AllGather", OP.bypass, replica_groups=rg,
                ins=[ag2_in[:].opt()],
                outs=[ag2_out[:].opt()])

            # ---- phase 4: GAT layer 2 + epilogue + output head ----
            for gi in range(NAG):
                h2cat = wp.tile([P, GRP, 128], F32, tag="h2cat")
                for j in range(GRP):
                    l = gi * GRP + j
                    g = ghat[l]
                    psB_t = ps.tile([P, 512], F32, tag="big")
                    psB = psB_t[:, 0:128]
                    den = wp.tile([P, 1], F32, tag="den2")
                    k0 = 0
                    while k0 < g:
                        kn = min(KC2, g - k0)
                        gt = gp2.tile([P, KC2, T2COLS], BF, tag="g2")
                        nc.gpsimd.dma_gather(
                            gt[:, 0:kn, :], ag2_out[:],
                            idx_sb[:, 8 * (int(goff[l]) + k0):
                                   8 * (int(goff[l]) + k0 + kn)],
                            P * kn, P * kn, T2COLS, single_packet=False,
                            queue_num=qrr[0] % NSWQ)
                        qrr[0] += 1
                        als_v = gt[:, 0:kn, 128:130].bitcast(F32)
                        u = wp.tile([P, KC2, 1], F32, tag="u2")
                        nc.vector.tensor_tensor(
                            out=u[:, 0:kn, :], in0=als_v,
                            in1=_bap(ald2[:, l, :], [(0, kn), (1, 1)]),
                            op=OP.add)
                        nc.vector.scalar_tensor_tensor(
                            out=u[:, 0:kn, :], in0=u[:, 0:kn, :], scalar=0.2,
                            in1=u[:, 0:kn, :], op0=OP.mult, op1=OP.max)
                        exf = wp.tile([P, KC2, 1], F32, tag="ex2")
                        dt_ = wp.tile([P, 1], F32, tag="dt2")
                        red = dt_ if k0 else den
                        nc.scalar.activation(exf[:, 0:kn, :], u[:, 0:kn, :],
                                             AF.Exp, accum_out=red[:])
                        if k0:
                            nc.vector.tensor_add(den[:], den[:], dt_[:])
                        exb = wp.tile([P, KC2, 1], BF, tag="exb2")
                        nc.vector.tensor_copy(out=exb[:, 0:kn, :],
                                              in_=exf[:, 0:kn, :])
                        nc.vector.tensor_tensor(
                            out=_bap(gt[:], [(T2COLS, kn), (1, 128)]),
                            in0=_bap(gt[:], [(T2COLS, kn), (1, 128)]),
                            in1=_bap(exb[:], [(1, kn), (0, 128)]),
                            op=OP.mult)
                        for k in range(kn):
                            nc.tensor.matmul(psB[:], lhsT=ident[:],
                                             rhs=gt[:, k, 0:128],
                                             start=(k0 + k == 0),
                                             stop=(k0 + k == g - 1))
                        k0 += kn
                    nc.vector.tensor_scalar_add(out=den[:], in0=den[:],
                                                scalar1=1e-30)
                    denr = wp.tile([P, 1], F32, tag="dr2")
                    nc.vector.reciprocal(denr[:], den[:])
                    nc.vector.scalar_tensor_tensor(
                        out=h2cat[:, j, :], in0=psB[:], scalar=denr[:],
                        in1=cc[:, CC_BIAS2:CC_BIAS2 + 128],
                        op0=OP.mult, op1=OP.add)

                # ---- group epilogue: LN2 + gelu + output head
                group_ln(h2cat, 128, cc[:, CC_G2:CC_G2 + 128],
                         cc[:, CC_B2:CC_B2 + 128], "ln2")
                h2b = wp.tile([P, GRP, 128], BF, tag="h2b")
                nc.scalar.activation(h2b[:], h2cat[:], AF.Gelu)
                for j in range(GRP):
                    l = gi * GRP + j
                    hoT = wp.tile([P, P], BF, tag="hoT")
                    transpose_to(hoT[:], h2b[:, j, :])
                    pso_t = pss.tile([P, 130], F32, tag="mm2")
                    pso = pso_t[:, 0:D_OUT]
                    nc.tensor.matmul(pso[:], lhsT=hoT[:], rhs=wos[:],
                                     start=True, stop=True)
                    z = wp.tile([P, D_OUT], F32, tag="z")
                    nc.vector.tensor_tensor(out=z[:], in0=pso[:],
                                            in1=cc[:, CC_BO:CC_BO + 32],
                                            op=OP.add)
                    m = wp.tile([P, 1], F32, tag="zm")
                    nc.vector.tensor_reduce(out=m[:], in_=z[:],
                                            axis=mybir.AxisListType.X,
                                            op=OP.max)
                    nc.vector.tensor_scalar_sub(out=zs[:, l, :], in0=z[:],
                                                scalar1=m[:])
                    ez = wp.tile([P, D_OUT], F32, tag="ez")
                    nc.scalar.activation(ez[:], zs[:, l, :], AF.Exp,
                                         accum_out=sdens[:, l:l + 1])

            # ---- log-softmax tail (batched: one Ln table load) ----
            nc.scalar.activation(lnds[:], sdens[:], AF.Ln)
            for gi in range(NAG):
                res = wp.tile([P, GRP, D_OUT], F32, tag="res")
                for j in range(GRP):
                    l = gi * GRP + j
                    nc.vector.tensor_scalar_sub(out=res[:, j, :],
                                                in0=zs[:, l, :],
                                                scalar1=lnds[:, l:l + 1])
                nc.sync.dma_start(
                    bass.AP(out.ap().tensor, gi * GRP * P * D_OUT,
                            [[D_OUT, P], [P * D_OUT, GRP], [1, D_OUT]]),
                    res[:])

    nc.compile()
    return nc


_CACHE = {}
_LAST_RUN = {}


def kernel(x, edge_index, g_in, b_in, W1, att1_s, att1_d, bias1, g1, b1,
           W2, att2_s, att2_d, bias2, g2, b2, Wo, bo):
    prep = prepare_inputs(x, edge_index)
    wts = prepare_weights(W1, att1_s, att1_d, bias1, g1, b1, g_in, b_in,
                          W2, att2_s, att2_d, bias2, g2, b2, Wo, bo,
                          prep["alsfix"])

    key = tuple(prep["ghat"])
    if key not in _CACHE:
        _CACHE[key] = build_program(prep["ghat"])
    nc = _CACHE[key]

    in_maps = []
    for c in range(NCORES):
        in_maps.append({
            "x_own": prep["x_own"][c],
            "idxw": prep["idxw"][c],
            "alsfix": wts["alsfix"][c],
            "w1ext": wts["w1ext"],
            "w2ext": wts["w2ext"].astype(BF16),
            "woext": wts["woext"],
            "colconst": wts["colconst"],
        })

    _LAST_RUN.update(nc=nc, in_maps=in_maps, prep=prep)
    res = bass_utils.run_bass_kernel_spmd(nc, in_maps,
                                          core_ids=list(range(NCORES)))
    outs = [res.results[c]["out"] for c in range(NCORES)]

    newid = prep["newid"]
    blk = newid // P
    core = blk % NCORES
    row = (blk // NCORES) * P + newid % P
    full = np.empty((N, D_OUT), dtype=np.float32)
    for c in range(NCORES):
        sel = core == c
        full[sel] = outs[c][row[sel]]
    return full


# revision 29
# speedup vs baseline: 1.1582x; 1.0364x over previous
"""Distributed 2-layer GAT kernel for 8 Trainium2 NeuronCores.

Strategy (host graph preprocessing + device SPMD kernel):
  * Nodes are relabeled by in-degree (ascending) and padded to 20480 ids.
    Blocks of 128 consecutive ids then have near-uniform in-degree, and the
    160 blocks are dealt round-robin to the 8 cores (core = block % 8), so
    every core sees the same per-block degree schedule ghat[l] (compile-time
    constant -> identical SPMD program; all per-core variation is in data).
  * Edge slots are dst-major: slot (block l, k, partition p) holds the k-th
    in-edge of dst p in block l.  A dma_gather pulls the k-th edge row of all
    128 dsts into one [128, cols] SBUF tile (row lands on its dst partition),
    so attention softmax needs no index math on device and the sum over
    in-edges is PSUM accumulation with a constant identity stationary matrix.
  * Each layer's per-node table row = [features bf16 | a_src f32] is built by
    the owning core and AllGather'd (Shared output scratchpad, only the used
    columns transferred) so gathers are core-local.
  * Layer-1 feature columns are stored head-interleaved ([c*4+h] instead of
    [h*128+c]) so the per-edge attention scaling is one tensor_tensor with
    step-1 innermost APs on both operands; W1/bias1/g1/b1 columns and W2 rows
    are permuted on the host to match.
  * LN gamma/beta of the input LN are folded into W1 / alsfix / bias1 on the
    host (exact).  Device LNs use ACT-engine Square+Identity with per-row
    scale/bias operands; per-block epilogues are batched in groups of 5 to
    amortize DVE op overhead and ACT table swaps.
  * Pad edge slots point to table row 0, whose a_src is forced to -1e9 on
    host data (alsfix), making exp(leaky_relu(...)) == 0 exactly.
"""
import sys

sys.path.insert(0, "/opt/trn_rl_repo")

import numpy as np
import ml_dtypes

from concourse import bass, bacc, tile, mybir
from concourse import bass_utils
from concourse.masks import make_identity

BF16 = ml_dtypes.bfloat16
F32 = mybir.dt.float32
BF = mybir.dt.bfloat16
I16 = mybir.dt.int16
AF = mybir.ActivationFunctionType
OP = mybir.AluOpType

# problem constants
N, E = 20000, 320000
D_IN, HID, D_OUT = 128, 128, 32
H1, H2 = 4, 1
EPS = 1e-5

NCORES = 8
P = 128
NPAD = 20480            # padded node count: 160 blocks of 128
NBLK_G = NPAD // P      # 160 global blocks
NPB = NPAD // NCORES    # 2560 nodes per core
NBLK = NPB // P         # 20 blocks per core
NEG = -1e9

T1COLS = 640            # L1 table row (bf16): 512 feats | 4 f32 a_src | pad
T1USED = 520            # columns actually written / AllGather'd
T2COLS = 256            # L2 table row (bf16): 128 feats | 1 f32 a_src | pad
T2USED = 130
KC1 = 16                # max in-edge slots per L1 gather call
KC2 = 16                # max in-edge slots per L2 gather call
NAG = 4                 # AllGather chunks (overlap with producer phase)
GRP = NBLK // NAG       # blocks per epilogue group / AG chunk
NSWQ = 4                # SWDGE queues for gather overlap

# colconst column layout (f32, each value replicated on all 128 partitions)
CC_G1, CC_B1, CC_BIAS1 = 0, 512, 1024
CC_G2, CC_B2, CC_BIAS2 = 1536, 1664, 1792
CC_BO = 1920
NCC = 1952

# head interleave: new L1 feature col c*H1+h <- old col h*HID+c
_PERM = (np.arange(H1 * HID).reshape(H1, HID).T).reshape(-1)  # new idx -> old idx


def _tid(n):
    """table row id of padded-node id n: single AllGather output is
    rank-major (rank c's shard occupies rows [c*NPB, (c+1)*NPB))."""
    blk = n // P
    c = blk % NCORES
    l = blk // NCORES
    return c * NPB + l * P + n % P


def prepare_inputs(x, edge_index):
    """Host graph preprocessing -> per-core arrays + degree schedule."""
    x = np.asarray(x, dtype=np.float32)
    ei = np.asarray(edge_index)
    src = np.concatenate([ei[0], np.arange(N, dtype=ei.dtype)]).astype(np.int64)
    dst = np.concatenate([ei[1], np.arange(N, dtype=ei.dtype)]).astype(np.int64)

    deg = np.bincount(dst, minlength=N)
    order = np.argsort(deg, kind="stable")        # orig node ids, deg ascending
    newid = np.empty(N, dtype=np.int64)           # orig -> padded id
    newid[order] = np.arange(N) + (NPAD - N)      # pads occupy ids [0, 480)

    degp = np.zeros(NPAD, dtype=np.int64)
    degp[newid] = deg
    gmax = degp.reshape(NBLK_G, P).max(axis=1)
    ghat = gmax.reshape(NBLK, NCORES).max(axis=1)         # per local block idx
    S = int(P * ghat.sum())                                # slots per core

    # CSR of in-edges keyed by new dst id
    nd = newid[dst]
    csr_order = np.argsort(nd, kind="stable")
    nsrc_sorted = newid[src[csr_order]]
    indptr = np.zeros(NPAD + 1, dtype=np.int64)
    np.cumsum(np.bincount(nd, minlength=NPAD), out=indptr[1:])

    tid_of = _tid(np.arange(NPAD))

    goff = np.zeros(NBLK, dtype=np.int64)                  # k-slot offsets
    goff[1:] = np.cumsum(ghat)[:-1]

    idxw = np.zeros((NCORES, P, S // 16), dtype=np.int16)
    x_own = np.zeros((NCORES, NPB, D_IN), dtype=np.float32)
    alsfix = np.zeros((NCORES, NPB, 8), dtype=np.float32)

    inv_new = np.full(NPAD, -1, dtype=np.int64)
    inv_new[newid] = np.arange(N)

    for c in range(NCORES):
        gblk = np.arange(NBLK) * NCORES + c                # global block ids
        nid = (gblk[:, None] * P + np.arange(P)).reshape(-1)   # [NPB] padded id
        ov = inv_new[nid]                                  # orig node or -1
        real = ov >= 0
        x_own[c][real] = x[ov[real]]
        alsfix[c][~real, :] = NEG

        idx_flat = np.zeros(S, dtype=np.int16)             # dummy -> row 0
        for l in range(NBLK):
            d0 = nid[l * P:(l + 1) * P]                    # padded ids of block
            base = goff[l] * P
            for p in range(P):
                d = d0[p]
                s0, s1 = indptr[d], indptr[d + 1]
                ks = np.arange(s1 - s0)
                idx_flat[base + ks * P + p] = tid_of[nsrc_sorted[s0:s1]]
        idxw[c] = np.tile(idx_flat.reshape(S // 16, 16).T, (NCORES, 1))

    return {
        "ghat": [int(g) for g in ghat],
        "S": S,
        "idxw": idxw,
        "x_own": x_own,
        "alsfix": alsfix,
        "newid": newid,
    }


def prepare_weights(W1, att1_s, att1_d, bias1, g1, b1, g_in, b_in,
                    W2, att2_s, att2_d, bias2, g2, b2, Wo, bo,
                    alsfix):
    W1 = np.asarray(W1, np.float64)
    W2 = np.asarray(W2, np.float64)
    a1s = np.asarray(att1_s, np.float64)
    a1d = np.asarray(att1_d, np.float64)

    # fold input-LN gamma/beta into W1:  LN0(x) = z*g_in + b_in  with
    # z = (x-mu)*rstd, so  LN0(x) @ W1 = z @ (diag(g_in) W1) + b_in @ W1.
    W1g = np.asarray(g_in, np.float64)[:, None] * W1           # [128, 512]
    c1 = np.asarray(b_in, np.float64) @ W1                     # [512]
    c1h = c1.reshape(H1, HID)
    # constant shift of attention logits per head (exact: u = als+ald+shift)
    ushift = np.einsum("hc,hc->h", c1h, a1s) + np.einsum("hc,hc->h", c1h, a1d)
    alsfix = alsfix.copy()
    alsfix[:, :, 0:4] += ushift[None, None, :]
    bias1f = np.asarray(bias1, np.float64) + c1                # sum(alpha)=1

    w1ext = np.zeros((D_IN, 520), dtype=BF16)
    w1ext[:, :512] = W1g[:, _PERM]                    # head-interleaved cols
    W1gh = W1g.reshape(D_IN, H1, HID)
    w1ext[:, 512:516] = np.einsum("khc,hc->kh", W1gh, a1s)
    w1ext[:, 516:520] = np.einsum("khc,hc->kh", W1gh, a1d)

    w2e = np.zeros((4 * HID, 130), dtype=np.float64)
    w2e[:, :128] = W2[_PERM, :]                       # rows follow interleave
    w2e[:, 128] = W2[_PERM, :] @ np.asarray(att2_s, np.float64)[0]
    w2e[:, 129] = W2[_PERM, :] @ np.asarray(att2_d, np.float64)[0]
    # pack [512, 130] -> [128, 4, 130] (partition p holds rows p, 128+p, ...)
    w2ext = np.ascontiguousarray(
        w2e.reshape(4, P, 130).transpose(1, 0, 2)).astype(BF16)

    woext = np.asarray(Wo, np.float32).astype(BF16)

    cc = np.zeros(NCC, dtype=np.float32)
    cc[CC_G1:CC_G1 + 512] = np.asarray(g1, np.float64)[_PERM]
    cc[CC_B1:CC_B1 + 512] = np.asarray(b1, np.float64)[_PERM]
    cc[CC_BIAS1:CC_BIAS1 + 512] = bias1f[_PERM]
    cc[CC_G2:CC_G2 + 128] = g2
    cc[CC_B2:CC_B2 + 128] = b2
    cc[CC_BIAS2:CC_BIAS2 + 128] = bias2
    cc[CC_BO:CC_BO + 32] = bo
    colconst = np.tile(cc[None, :], (P, 1))

    return {"w1ext": w1ext, "w2ext": w2ext.reshape(P, 4 * 130),
            "woext": woext, "colconst": colconst, "alsfix": alsfix}


def _bap(ap, dims):
    """AP with explicit free-dim [step, count] pairs (partition dim kept)."""
    return bass.AP(ap.tensor, ap.offset, [ap.ap[0]] + [list(d) for d in dims])


def build_program(ghat, num_devices=NCORES):
    S = int(P * sum(ghat))
    goff = np.zeros(NBLK, dtype=np.int64)
    goff[1:] = np.cumsum(ghat)[:-1]

    nc = bacc.Bacc("TRN2", target_bir_lowering=False, debug=False,
                   num_devices=num_devices, num_swdge_queues=NSWQ)

    x_own = nc.dram_tensor("x_own", [NPB, D_IN], F32, kind="ExternalInput")
    idxw = nc.dram_tensor("idxw", [P, S // 16], I16, kind="ExternalInput")
    alsfix = nc.dram_tensor("alsfix", [NPB, 8], F32, kind="ExternalInput")
    w1ext = nc.dram_tensor("w1ext", [D_IN, 520], BF, kind="ExternalInput")
    w2ext = nc.dram_tensor("w2ext", [P, 4 * 130], BF, kind="ExternalInput")
    woext = nc.dram_tensor("woext", [P, D_OUT], BF, kind="ExternalInput")
    colconst = nc.dram_tensor("colconst", [P, NCC], F32, kind="ExternalInput")
    out = nc.dram_tensor("out", [NPB, D_OUT], F32, kind="ExternalOutput")

    rg = [list(range(num_devices))]
    qrr = [0]

    with tile.TileContext(nc) as tc:
        with (
            tc.tile_pool(name="cst", bufs=1) as cst,
            tc.tile_pool(name="wp", bufs=2) as wp,
            tc.tile_pool(name="gp", bufs=4) as gp,
            tc.tile_pool(name="gp2", bufs=4) as gp2,
            tc.tile_pool(name="ps", bufs=2, space="PSUM") as ps,
            tc.tile_pool(name="pss", bufs=2, space="PSUM") as pss,
            tc.tile_pool(name="dram", bufs=1, space="DRAM") as dram,
        ):
            # ---- constants ----
            ident = cst.tile([P, P], BF)
            make_identity(nc, ident[:])
            w1s = cst.tile([P, 520], BF)
            nc.sync.dma_start(w1s[:], w1ext[:])
            w2s = cst.tile([P, 4, 130], BF)
            nc.sync.dma_start(w2s[:], w2ext[:])
            wos = cst.tile([P, D_OUT], BF)
            nc.sync.dma_start(wos[:], woext[:])
            cc = cst.tile([P, NCC], F32)
            nc.sync.dma_start(cc[:], colconst[:])
            idx_sb = cst.tile([P, S // 16], I16)
            nc.sync.dma_start(idx_sb[:], idxw[:])
            afix = cst.tile([P, NBLK, 8], F32)
            nc.sync.dma_start(
                afix[:], bass.AP(alsfix.ap().tensor, 0,
                                 [[8, P], [8 * P, NBLK], [1, 8]]))
            eps_t = cst.tile([P, 1], F32)
            nc.vector.memset(eps_t[:], EPS)
            ald1 = cst.tile([P, NBLK, H1], F32)
            ald2 = cst.tile([P, NBLK, 1], F32)
            zs = cst.tile([P, NBLK, D_OUT], F32)
            sdens = cst.tile([P, NBLK], F32)
            lnds = cst.tile([P, NBLK], F32)

            ag1_in = dram.tile([NPB, T1COLS], BF)
            ag1_out = dram.tile([NPAD, T1COLS], BF, addr_space="Shared")
            ag2_in = dram.tile([NPB, T2COLS], BF)
            ag2_out = dram.tile([NPAD, T2COLS], BF, addr_space="Shared")

            def transpose_to(dst_bf, src_bf):
                pst = pss.tile([P, P], BF, tag="tp")
                nc.tensor.transpose(out=pst[:], in_=src_bf, identity=ident[:])
                nc.vector.tensor_copy(out=dst_bf, in_=pst[:])

            def group_ln(hcat, ncols, cg, cb, tag):
                """Batched LayerNorm over a [P, GRP, ncols] f32 tile, in
                place: hcat <- LN(hcat) * g + b, then caller applies gelu."""
                sq = wp.tile([P, ncols], F32, tag=f"{tag}_sq")
                ssq = wp.tile([P, GRP], F32, tag=f"{tag}_ssq")
                for j in range(GRP):
                    nc.scalar.activation(sq[:], hcat[:, j, :], AF.Square,
                                         accum_out=ssq[:, j:j + 1])
                msum = wp.tile([P, GRP], F32, tag=f"{tag}_ms")
                nc.vector.tensor_reduce(out=msum[:], in_=hcat[:],
                                        axis=mybir.AxisListType.X, op=OP.add)
                mu = wp.tile([P, GRP], F32, tag=f"{tag}_mu")
                nc.vector.tensor_scalar_mul(out=mu[:], in0=msum[:],
                                            scalar1=1.0 / ncols)
                mu2 = wp.tile([P, GRP], F32, tag=f"{tag}_m2")
                nc.vector.tensor_mul(mu2[:], mu[:], mu[:])
                var = wp.tile([P, GRP], F32, tag=f"{tag}_va")
                nc.vector.scalar_tensor_tensor(
                    out=var[:], in0=ssq[:], scalar=1.0 / ncols, in1=mu2[:],
                    op0=OP.mult, op1=OP.subtract)
                sd = wp.tile([P, GRP], F32, tag=f"{tag}_sd")
                nc.scalar.activation(sd[:], var[:], AF.Sqrt, bias=eps_t[:])
                rstd = wp.tile([P, GRP], F32, tag=f"{tag}_rs")
                nc.vector.reciprocal(rstd[:], sd[:])
                nmr = wp.tile([P, GRP], F32, tag=f"{tag}_nm")
                nc.vector.scalar_tensor_tensor(
                    out=nmr[:], in0=mu[:], scalar=-1.0, in1=rstd[:],
                    op0=OP.mult, op1=OP.mult)
                for j in range(GRP):
                    nc.scalar.activation(hcat[:, j, :], hcat[:, j, :],
                                         AF.Identity, bias=nmr[:, j:j + 1],
                                         scale=rstd[:, j:j + 1])
                nc.vector.tensor_tensor(
                    out=hcat[:], in0=hcat[:],
                    in1=_bap(cg, [(0, GRP), (1, ncols)]), op=OP.mult)
                nc.vector.tensor_tensor(
                    out=hcat[:], in0=hcat[:],
                    in1=_bap(cb, [(0, GRP), (1, ncols)]), op=OP.add)

            # ---- phase 0: LN0 (folded gamma/beta) + W1 matmul ----
            for gi in range(NAG):
                xt = wp.tile([P, GRP, D_IN], F32, tag="xt")
                nc.sync.dma_start(
                    xt[:], bass.AP(x_own.ap().tensor,
                                   gi * GRP * P * D_IN,
                                   [[D_IN, P], [P * D_IN, GRP], [1, D_IN]]))
                sq0 = wp.tile([P, GRP, D_IN], F32, tag="sq0")
                nc.vector.tensor_mul(sq0[:], xt[:], xt[:])
                ssq0 = wp.tile([P, GRP], F32, tag="ssq0")
                nc.vector.tensor_reduce(out=ssq0[:], in_=sq0[:],
                                        axis=mybir.AxisListType.X, op=OP.add)
                ms0 = wp.tile([P, GRP], F32, tag="ms0")
                nc.vector.tensor_reduce(out=ms0[:], in_=xt[:],
                                        axis=mybir.AxisListType.X, op=OP.add)
                mu0 = wp.tile([P, GRP], F32, tag="mu0")
                nc.vector.tensor_scalar_mul(out=mu0[:], in0=ms0[:],
                                            scalar1=1.0 / D_IN)
                mu20 = wp.tile([P, GRP], F32, tag="mu20")
                nc.vector.tensor_mul(mu20[:], mu0[:], mu0[:])
                var0 = wp.tile([P, GRP], F32, tag="var0")
                nc.vector.scalar_tensor_tensor(
                    out=var0[:], in0=ssq0[:], scalar=1.0 / D_IN, in1=mu20[:],
                    op0=OP.mult, op1=OP.subtract)
                sd0 = wp.tile([P, GRP], F32, tag="sd0")
                nc.scalar.activation(sd0[:], var0[:], AF.Sqrt, bias=eps_t[:])
                rs0 = wp.tile([P, GRP], F32, tag="rs0")
                nc.vector.reciprocal(rs0[:], sd0[:])
                xc0 = wp.tile([P, GRP, D_IN], F32, tag="xc0")
                nc.vector.tensor_tensor(
                    out=xc0[:], in0=xt[:],
                    in1=_bap(mu0[:], [(1, GRP), (0, D_IN)]), op=OP.subtract)
                xnb = wp.tile([P, GRP, D_IN], BF, tag="xnb")
                nc.vector.tensor_tensor(
                    out=xnb[:], in0=xc0[:],
                    in1=_bap(rs0[:], [(1, GRP), (0, D_IN)]), op=OP.mult)
                for j in range(GRP):
                    t = gi * GRP + j
                    xT = wp.tile([P, P], BF, tag="xT")
                    transpose_to(xT[:], xnb[:, j, :])
                    ps1 = ps.tile([P, 512], F32, tag="big")
                    nc.tensor.matmul(ps1[:], lhsT=xT[:], rhs=w1s[:, 0:512],
                                     start=True, stop=True)
                    ps2_t = pss.tile([P, 130], F32, tag="mm2")
                    ps2 = ps2_t[:, 0:8]
                    nc.tensor.matmul(ps2[:], lhsT=xT[:], rhs=w1s[:, 512:520],
                                     start=True, stop=True)
                    tt = wp.tile([P, T1USED], BF, tag="tt")
                    nc.vector.tensor_copy(out=tt[:, 0:512], in_=ps1[:])
                    nc.vector.tensor_tensor(
                        out=tt[:, 512:520].bitcast(F32), in0=ps2[:, 0:4],
                        in1=afix[:, t, 0:4], op=OP.add)
                    nc.vector.tensor_copy(out=ald1[:, t, :], in_=ps2[:, 4:8])
                    nc.sync.dma_start(ag1_in[t * P:(t + 1) * P, 0:T1USED],
                                      tt[:])

            # ---- AllGather the L1 table (Shared output; collectives need
            # contiguous APs so the pad columns ride along)
            nc.gpsimd.collective_compute(
                "AllGather", OP.bypass, replica_groups=rg,
                ins=[ag1_in[:].opt()],
                outs=[ag1_out[:].opt()])

            # ---- phase 2: GAT layer 1 + epilogue + W2 matmul ----
            for gi in range(NAG):
                h1cat = wp.tile([P, GRP, 512], F32, tag="h1cat")
                for j in range(GRP):
                    l = gi * GRP + j
                    g = ghat[l]
                    psA = ps.tile([P, 512], F32, tag="big")
                    den = wp.tile([P, H1], F32, tag="den1")
                    k0 = 0
                    while k0 < g:
                        kn = min(KC1, g - k0)
                        gt = gp.tile([P, KC1, T1COLS], BF, tag="g1")
                        nc.gpsimd.dma_gather(
                            gt[:, 0:kn, :], ag1_out[:],
                            idx_sb[:, 8 * (int(goff[l]) + k0):
                                   8 * (int(goff[l]) + k0 + kn)],
                            P * kn, P * kn, T1COLS, single_packet=False,
                            queue_num=qrr[0] % NSWQ)
                        qrr[0] += 1
                        als_v = gt[:, 0:kn, 512:520].bitcast(F32)
                        u = wp.tile([P, KC1, H1], F32, tag="u1")
                        nc.vector.tensor_tensor(
                            out=u[:, 0:kn, :], in0=als_v,
                            in1=_bap(ald1[:, l, :], [(0, kn), (1, H1)]),
                            op=OP.add)
                        nc.vector.scalar_tensor_tensor(
                            out=u[:, 0:kn, :], in0=u[:, 0:kn, :], scalar=0.2,
                            in1=u[:, 0:kn, :], op0=OP.mult, op1=OP.max)
                        exf = wp.tile([P, KC1, H1], F32, tag="ex1")
                        nc.scalar.activation(exf[:, 0:kn, :], u[:, 0:kn, :],
                                             AF.Exp)
                        dt_ = wp.tile([P, H1], F32, tag="dt1")
                        red = dt_ if k0 else den
                        nc.vector.tensor_reduce(
                            out=red[:], in_=_bap(exf[:], [(1, H1), (H1, kn)]),
                            axis=mybir.AxisListType.X, op=OP.add)
                        if k0:
                            nc.vector.tensor_add(den[:], den[:], dt_[:])
                        exb = wp.tile([P, KC1, H1], BF, tag="exb1")
                        nc.vector.tensor_copy(out=exb[:, 0:kn, :],
                                              in_=exf[:, 0:kn, :])
                        # in-place scale of the gathered features; columns are
                        # head-interleaved so all steps are +/-1 or outer
                        nc.vector.tensor_tensor(
                            out=_bap(gt[:], [(T1COLS, kn), (H1, HID),
                                             (1, H1)]),
                            in0=_bap(gt[:], [(T1COLS, kn), (H1, HID),
                                             (1, H1)]),
                            in1=_bap(exb[:], [(H1, kn), (0, HID), (1, H1)]),
                            op=OP.mult)
                        for k in range(kn):
                            nc.tensor.matmul(psA[:], lhsT=ident[:],
                                             rhs=gt[:, k, 0:512],
                                             start=(k0 + k == 0),
                                             stop=(k0 + k == g - 1))
                        k0 += kn
                    nc.vector.tensor_scalar_add(out=den[:], in0=den[:],
                                                scalar1=1e-30)
                    denr = wp.tile([P, H1], F32, tag="dr1")
                    nc.vector.reciprocal(denr[:], den[:])
                    # h1 = psA * denr(per head, interleaved bcast) + bias1
                    nc.vector.tensor_tensor(
                        out=_bap(h1cat[:, j, :], [(H1, HID), (1, H1)]),
                        in0=_bap(psA[:], [(H1, HID), (1, H1)]),
                        in1=_bap(denr[:], [(0, HID), (1, H1)]),
                        op=OP.mult)
                    nc.vector.tensor_tensor(
                        out=h1cat[:, j, :], in0=h1cat[:, j, :],
                        in1=cc[:, CC_BIAS1:CC_BIAS1 + 512], op=OP.add)

                # ---- group epilogue: LN1 + gelu (batched over GRP blocks)
                group_ln(h1cat, 512, cc[:, CC_G1:CC_G1 + 512],
                         cc[:, CC_B1:CC_B1 + 512], "ln1")
                h1b = wp.tile([P, GRP, 512], BF, tag="h1b")
                nc.scalar.activation(h1b[:], h1cat[:], AF.Gelu)
                for j in range(GRP):
                    l = gi * GRP + j
                    ps3 = pss.tile([P, 130], F32, tag="mm2")
                    for cch in range(4):
                        hT = wp.tile([P, P], BF, tag="hT")
                        transpose_to(hT[:], h1b[:, j, cch * P:(cch + 1) * P])
                        nc.tensor.matmul(ps3[:], lhsT=hT[:], rhs=w2s[:, cch, :],
                                         start=(cch == 0), stop=(cch == 3))
                    t2 = wp.tile([P, T2USED], BF, tag="t2")
                    nc.vector.tensor_copy(out=t2[:, 0:128], in_=ps3[:, 0:128])
                    nc.vector.tensor_tensor(
                        out=t2[:, 128:130].bitcast(F32), in0=ps3[:, 128:129],
                        in1=afix[:, l, 4:5], op=OP.add)
                    nc.vector.tensor_copy(out=ald2[:, l, :],
                                          in_=ps3[:, 129:130])
                    nc.sync.dma_start(ag2_in[l * P:(l + 1) * P, 0:T2USED],
                                      t2[:])

            # ---- AllGather the L2 table (Shared output)
            nc.gpsimd.collective_compute(
                "AllGather", OP.bypass, replica_groups=rg,
                ins=[ag2_in[:].opt()],
                outs=[ag2_out[:].opt()])

            # ---- phase 4: GAT layer 2 + epilogue + output head ----
            for gi in range(NAG):
                h2cat = wp.tile([P, GRP, 128], F32, tag="h2cat")
                for j in range(GRP):
                    l = gi * GRP + j
                    g = ghat[l]
                    psB_t = ps.tile([P, 512], F32, tag="big")
                    psB = psB_t[:, 0:128]
                    den = wp.tile([P, 1], F32, tag="den2")
                    k0 = 0
                    while k0 < g:
                        kn = min(KC2, g - k0)
                        gt = gp2.tile([P, KC2, T2COLS], BF, tag="g2")
                        nc.gpsimd.dma_gather(
                            gt[:, 0:kn, :], ag2_out[:],
                            idx_sb[:, 8 * (int(goff[l]) + k0):
                                   8 * (int(goff[l]) + k0 + kn)],
                            P * kn, P * kn, T2COLS, single_packet=False,
                            queue_num=qrr[0] % NSWQ)
                        qrr[0] += 1
                        als_v = gt[:, 0:kn, 128:130].bitcast(F32)
                        u = wp.tile([P, KC2, 1], F32, tag="u2")
                        nc.vector.tensor_tensor(
                            out=u[:, 0:kn, :], in0=als_v,
                            in1=_bap(ald2[:, l, :], [(0, kn), (1, 1)]),
                            op=OP.add)
                        nc.vector.scalar_tensor_tensor(
                            out=u[:, 0:kn, :], in0=u[:, 0:kn, :], scalar=0.2,
                            in1=u[:, 0:kn, :], op0=OP.mult, op1=OP.max)
                        exf = wp.tile([P, KC2, 1], F32, tag="ex2")
                        dt_ = wp.tile([P, 1], F32, tag="dt2")
                        red = dt_ if k0 else den
                        nc.scalar.activation(exf[:, 0:kn, :], u[:, 0:kn, :],
                                             AF.Exp, accum_out=red[:])
                        if k0:
                            nc.vector.tensor_add(den[:], den[:], dt_[:])
                        exb = wp.tile([P, KC2, 1], BF, tag="exb2")
                        nc.vector.tensor_copy(out=exb[:, 0:kn, :],
                                              in_=exf[:, 0:kn, :])
                        nc.vector.tensor_tensor(
                            out=_bap(gt[:], [(T2COLS, kn), (1, 128)]),
                            in0=_bap(gt[:], [(T2COLS, kn), (1, 128)]),
                            in1=_bap(exb[:], [(1, kn), (0, 128)]),
                            op=OP.mult)
                        for k in range(kn):
                            nc.tensor.matmul(psB[:], lhsT=ident[:],
                                             rhs=gt[:, k, 0:128],
                                             start=(k0 + k == 0),
                                             stop=(k0 + k == g - 1))
                        k0 += kn
                    nc.vector.tensor_scalar_add(out=den[:], in0=den[:],
                                                scalar1=1e-30)
                    denr = wp.tile([P, 1], F32, tag="dr2")
                    nc.vector.reciprocal(denr[:], den[:])
                    nc.vector.scalar_tensor_tensor(
                        out=h2cat[:, j, :], in0=psB[:], scalar=denr[:],
                        in1=cc[:, CC_BIAS2:CC_BIAS2 + 128],
                        op0=OP.mult, op1=OP.add)

                # ---- group epilogue: LN2 + gelu + output head
                group_ln(h2cat, 128, cc[:, CC_G2:CC_G2 + 128],
                         cc[:, CC_B2:CC_B2 + 128], "ln2")
                h2b = wp.tile([P, GRP, 128], BF, tag="h2b")
                nc.scalar.activation(h2b[:], h2cat[:], AF.Gelu)
                for j in range(GRP):
                    l = gi * GRP + j
                    hoT = wp.tile([P, P], BF, tag="hoT")
                    transpose_to(hoT[:], h2b[:, j, :])
                    pso_t = pss.tile([P, 130], F32, tag="mm2")
                    pso = pso_t[:, 0:D_OUT]
                    nc.tensor.matmul(pso[:], lhsT=hoT[:], rhs=wos[:],
                                     start=True, stop=True)
                    z = wp.tile([P, D_OUT], F32, tag="z")
                    nc.vector.tensor_tensor(out=z[:], in0=pso[:],
                                            in1=cc[:, CC_BO:CC_BO + 32],
                                            op=OP.add)
                    m = wp.tile([P, 1], F32, tag="zm")
                    nc.vector.tensor_reduce(out=m[:], in_=z[:],
                                            axis=mybir.AxisListType.X,
                                            op=OP.max)
                    nc.vector.tensor_scalar_sub(out=zs[:, l, :], in0=z[:],
                                                scalar1=m[:])
                    ez = wp.tile([P, D_OUT], F32, tag="ez")
                    nc.scalar.activation(ez[:], zs[:, l, :], AF.Exp,
                                         accum_out=sdens[:, l:l + 1])

            # ---- log-softmax tail (batched: one Ln table load) ----
            nc.scalar.activation(lnds[:], sdens[:], AF.Ln)
            for gi in range(NAG):
                res = wp.tile([P, GRP, D_OUT], F32, tag="res")
                for j in range(GRP):
                    l = gi * GRP + j
                    nc.vector.tensor_scalar_sub(out=res[:, j, :],
                                                in0=zs[:, l, :],
                                                scalar1=lnds[:, l:l + 1])
                nc.sync.dma_start(
                    bass.AP(out.ap().tensor, gi * GRP * P * D_OUT,
                            [[D_OUT, P], [P * D_OUT, GRP], [1, D_OUT]]),
                    res[:])

    nc.compile()
    return nc


_CACHE = {}
_LAST_RUN = {}


def kernel(x, edge_index, g_in, b_in, W1, att1_s, att1_d, bias1, g1, b1,
           W2, att2_s, att2_d, bias2, g2, b2, Wo, bo):
    prep = prepare_inputs(x, edge_index)
    wts = prepare_weights(W1, att1_s, att1_d, bias1, g1, b1, g_in, b_in,
                          W2, att2_s, att2_d, bias2, g2, b2, Wo, bo,
                          prep["alsfix"])

    key = tuple(prep["ghat"])
    if key not in _CACHE:
        _CACHE[key] = build_program(prep["ghat"])
    nc = _CACHE[key]

    in_maps = []
    for c in range(NCORES):
        in_maps.append({
            "x_own": prep["x_own"][c],
            "idxw": prep["idxw"][c],
            "alsfix": wts["alsfix"][c],
            "w1ext": wts["w1ext"],
            "w2ext": wts["w2ext"].astype(BF16),
            "woext": wts["woext"],
            "colconst": wts["colconst"],
        })

    _LAST_RUN.update(nc=nc, in_maps=in_maps, prep=prep)
    res = bass_utils.run_bass_kernel_spmd(nc, in_maps,
                                          core_ids=list(range(NCORES)))
    outs = [res.results[c]["out"] for c in range(NCORES)]

    newid = prep["newid"]
    blk = newid // P
    core = blk % NCORES
    row = (blk // NCORES) * P + newid % P
    full = np.empty((N, D_OUT), dtype=np.float32)
    for c in range(NCORES):
        sel = core == c
        full[sel] = outs[c][row[sel]]
    return full
